# revision 4
# baseline (speedup 1.0000x reference)
"""MeanNSE (segment-reduce) Trainium2 kernel — 8 NeuronCores, data-parallel.

v2: PE-array segment reduction. The basin ids are pure index data, so all
index math runs on the host; the device does every FLOP over the 16.7M
float arrays.

Host: stable-sort elements by basin and pack them into per-core [128, C]
(C = 17408 = 34*512) bf16 tiles in "pillar slot" layout: slot s in [0,512)
owns the 4352 elements at positions {(p, g*512 + s) : p<128, g<34}; each
basin is padded (with zeros) to a whole number of slots, so every slot
contains elements of exactly one basin.  Slot sums can then be computed on
the TENSOR engine: a [128,1] ones stationary x [128,512] moving matmul
yields all 512 per-column partition-sums of one 512-col chunk, and the 34
chunk matmuls accumulate in one PSUM bank (per-element has_written logic),
producing Sum over each slot's full pillar.

Device (per core), three stats per slot, engines balanced:
  - TENSOR: 3 stat passes x 34 accumulating matmuls (N=512, ones
    stationary, ~0.42 ns/col) into 3 psum banks; ~40 warm-up matmuls into a
    scratch bank during the initial DMA keep the PE HAM clock at 2.4 GHz.
  - DVE:  d = t - p, d2 = d*d   (tensor_tensor bf16 runs in 2x mode)
  - ACT:  t2 = Square(t)        (activation, 1x)
  - DMA: inputs stream over sync + scalar HW DGE queues and the gpsimd
    software queue, byte-balanced (~3MB each), small tiles first/last.
Outputs are just 3x512 f32 slot sums -> psum drained via DVE/ACT copies
-> one tiny DMA out.

Host: bincount slot sums back to basins (slot->basin map is host data),
combine in float64 with exact integer counts:
  ss_tot = sum_t2 - sum_t^2/count, nse = 1 - ss_res/(ss_tot + 1e-10),
  answer = mean over 671 basins.
"""

import sys

sys.path.insert(0, "/opt/trn_rl_repo")

import numpy as np
import ml_dtypes

import concourse.bacc as bacc
import concourse.mybir as mybir
import concourse.tile as tile
from concourse.bass_utils import run_bass_kernel_spmd

F32 = mybir.dt.float32
BF16 = mybir.dt.bfloat16
BF16_NP = ml_dtypes.bfloat16

N_CORES = 8
N_TOTAL = 16777216
N_BASINS = 671
EPS = 1e-10

P = 128  # partitions
CH = 512  # psum bank width (f32) = matmul N
SUP = 34  # chunks per stat pass
C = SUP * CH  # columns per core (17408)
E_C = P * C  # elements per core (2,228,224)
PILLAR = P * SUP  # elements per slot (4352)
SLOTS = CH  # slots per core (512)
U_TOT = N_CORES * SLOTS  # global slot-units (4096)

# DMA tile plan, in 512-col chunks (sums to SUP=34): small tiles first so
# compute starts early, small tiles last so the tail drains fast.
K_PLAN = [1, 1, 2, 4, 4, 4, 4, 4, 4, 4, 1, 1]
N_WARM = 6  # PE warm-up matmuls bridging preamble -> first data

_AF = mybir.ActivationFunctionType

_cache = {}


def _dma_schedule():
    """Tile-major round-robin: the sim's DMA device is a single serialized
    resource ordered by trigger time, so interleaving (yt0, yp0, yt1, ...)
    across the three trigger engines makes tiles land in exact tile order,
    with each tile's yt/yp adjacent."""
    qnames = ["sync", "scalar", "gpsimd"]
    sched = {q: [] for q in qnames}
    order = []
    for t in range(len(K_PLAN)):
        order.append(("yt", t))
        order.append(("yp", t))
    for i, pair in enumerate(order):
        sched[qnames[i % 3]].append(pair)
    return sched


def _build():
    nc = bacc.Bacc()
    yt = nc.declare_dram_parameter("yt", [E_C], BF16, isOutput=False)
    yp = nc.declare_dram_parameter("yp", [E_C], BF16, isOutput=False)
    # out: [sum_t(512) | sum_t2(512) | sum_d2(512)]
    out = nc.declare_dram_parameter("out", [3 * SLOTS], F32, isOutput=True)

    yt2d = yt[:].rearrange("(p c) -> p c", p=P, c=C)
    yp2d = yp[:].rearrange("(p c) -> p c", p=P, c=C)

    sched = _dma_schedule()

    with tile.TileContext(nc) as tc:
        with (
            tc.tile_pool(name="const", bufs=1) as cpool,
            tc.tile_pool(name="io", bufs=1) as io_pool,
            tc.tile_pool(name="dx", bufs=2) as d_pool,
            tc.tile_pool(name="d2x", bufs=2) as d2_pool,
            tc.tile_pool(name="t2x", bufs=2) as t2_pool,
            tc.tile_pool(name="ps", bufs=1, space="PSUM") as psum_pool,
        ):
            ones = cpool.tile([P, 1], BF16, tag="ones")
            warm = cpool.tile([P, CH], BF16, tag="warm")
            outs = cpool.tile([1, 3 * SLOTS], F32, tag="outs")
            nc.vector.memset(ones[:, :], 1.0)
            nc.vector.memset(warm[:, :], 0.0)

            p_t = psum_pool.tile([1, CH], F32, tag="p_t")
            p_t2 = psum_pool.tile([1, CH], F32, tag="p_t2")
            p_d2 = psum_pool.tile([1, CH], F32, tag="p_d2")
            p_w = psum_pool.tile([1, CH], F32, tag="p_w")

            # PE warm-up: keep the HAM activity monitor busy during the
            # initial DMA so real matmuls run at 2.4 GHz.
            for w in range(N_WARM):
                nc.tensor.matmul(
                    p_w[:, :], ones[:, :], warm[:, :],
                    start=(w == 0), stop=(w == N_WARM - 1),
                )

            # stage all input tiles up front across the three DMA queues
            tiles = []
            base = 0
            for t, k in enumerate(K_PLAN):
                tt_ = io_pool.tile([P, k * CH], BF16, tag=f"yt{t}")
                tp_ = io_pool.tile([P, k * CH], BF16, tag=f"yp{t}")
                tiles.append((tt_, tp_, k, base))
                base += k * CH

            def _dst(arr, t):
                return tiles[t][0 if arr == "yt" else 1][:, :]

            def _src(arr, t):
                b, k = tiles[t][3], tiles[t][2]
                src = yt2d if arr == "yt" else yp2d
                return src[:, b : b + k * CH]

            for arr, t in sched["sync"]:
                nc.sync.dma_start(_dst(arr, t), _src(arr, t))
            for arr, t in sched["scalar"]:
                nc.scalar.dma_start(_dst(arr, t), _src(arr, t))
            for arr, t in sched["gpsimd"]:
                nc.gpsimd.dma_start(_dst(arr, t), _src(arr, t))

            # PE consumes the quadratic stats one tile behind the raw-t
            # stream, so it never head-of-line blocks on DVE/ACT results.
            def _mm(psum, src, cg0, k):
                for c in range(k):
                    nc.tensor.matmul(
                        psum[:, :], ones[:, :], src[:, c * CH : (c + 1) * CH],
                        start=(cg0 + c == 0), stop=(cg0 + c == SUP - 1),
                    )

            lagged = []  # (t2_tile, d2_tile, cg0, k) awaiting PE
            cg = 0
            for tt_, tp_, k, base in tiles:
                # raw-t slot sums: only need yt, keep PE fed right away
                _mm(p_t, tt_, cg, k)
                # t^2 on ACT (depends only on yt)
                t2_t = t2_pool.tile([P, k * CH], BF16, tag="t2")
                nc.scalar.activation(t2_t[:, :], tt_[:, :], _AF.Square)
                # (t-p)^2 on DVE (both tensor_tensor ops run 2x in bf16)
                d_t = d_pool.tile([P, k * CH], BF16, tag="d")
                nc.vector.tensor_sub(d_t[:, :], tt_[:, :], tp_[:, :])
                d2_t = d2_pool.tile([P, k * CH], BF16, tag="d2")
                nc.vector.tensor_mul(d2_t[:, :], d_t[:, :], d_t[:, :])
                lagged.append((t2_t, d2_t, cg, k))
                if len(lagged) > 1:
                    lt2, ld2, lcg, lk = lagged.pop(0)
                    _mm(p_t2, lt2, lcg, lk)
                    _mm(p_d2, ld2, lcg, lk)
                cg += k

            # p_t is complete: drain it on ACT while PE flushes the last tile
            nc.scalar.activation(outs[:, 0:CH], p_t[:, :], _AF.Copy)
            lt2, ld2, lcg, lk = lagged.pop(0)
            _mm(p_d2, ld2, lcg, lk)
            _mm(p_t2, lt2, lcg, lk)
            # drain remaining psum banks on two engines in parallel
            nc.vector.tensor_copy(outs[:, 2 * CH : 3 * CH], p_d2[:, :])
            nc.scalar.activation(outs[:, CH : 2 * CH], p_t2[:, :], _AF.Copy)

            nc.sync.dma_start(
                out[:].rearrange("(p x) -> p x", p=1, x=3 * SLOTS),
                outs[:, :],
            )
    nc.compile()
    return nc


def _get_nc():
    if "nc" not in _cache:
        _cache["nc"] = _build()
    return _cache["nc"]


def _prepare(y_pred, y_true, basin):
    """Host-side index math: sort by basin, pack into pillar-slot layout."""
    y_pred = np.asarray(y_pred, dtype=np.float32)
    y_true = np.asarray(y_true, dtype=np.float32)
    b = np.asarray(basin).astype(np.int32)
    n = b.shape[0]

    counts = np.bincount(b, minlength=N_BASINS)
    m = (counts + PILLAR - 1) // PILLAR  # slots per basin
    u_tot = int(m.sum())
    assert u_tot <= U_TOT, (u_tot, U_TOT)
    base_u = np.zeros(N_BASINS + 1, np.int64)
    np.cumsum(m, out=base_u[1:])

    order = np.argsort(b, kind="stable")
    seg_start = np.zeros(N_BASINS, np.int64)
    np.cumsum(counts[:-1], out=seg_start[1:])
    bs = b[order]
    i_local = np.arange(n, dtype=np.int64) - seg_start[bs]
    su = base_u[bs] + i_local // PILLAR  # global slot-unit
    j = i_local % PILLAR
    p = j // SUP
    g = j % SUP
    core = su // SLOTS
    s = su % SLOTS
    dst = core * E_C + p * C + g * CH + s

    yt_pad = np.zeros(N_CORES * E_C, dtype=BF16_NP)
    yp_pad = np.zeros(N_CORES * E_C, dtype=BF16_NP)
    yt_pad[dst] = y_true[order].astype(BF16_NP)
    yp_pad[dst] = y_pred[order].astype(BF16_NP)
    yt_pad = yt_pad.reshape(N_CORES, E_C)
    yp_pad = yp_pad.reshape(N_CORES, E_C)

    in_maps = [{"yt": yt_pad[c], "yp": yp_pad[c]} for c in range(N_CORES)]

    # basin of every global slot-unit (pad units -> N_BASINS, dropped later)
    slot_basin = np.full(U_TOT, N_BASINS, np.int64)
    slot_basin[:u_tot] = np.repeat(np.arange(N_BASINS), m)
    return in_maps, (counts, slot_basin)


def _finish(results, ctx):
    counts, slot_basin = ctx
    sums = np.empty((3, U_TOT), np.float64)
    for c in range(N_CORES):
        arr = np.asarray(results[c]["out"], np.float64).reshape(3, SLOTS)
        sums[:, c * SLOTS : (c + 1) * SLOTS] = arr
    s_t = np.bincount(slot_basin, weights=sums[0], minlength=N_BASINS + 1)[:N_BASINS]
    s_t2 = np.bincount(slot_basin, weights=sums[1], minlength=N_BASINS + 1)[:N_BASINS]
    s_d2 = np.bincount(slot_basin, weights=sums[2], minlength=N_BASINS + 1)[:N_BASINS]
    cnt = counts.astype(np.float64)
    ss_tot = s_t2 - s_t * s_t / cnt
    nse = 1.0 - s_d2 / (ss_tot + EPS)
    return np.float32(nse.mean())


def kernel(y_pred, y_true, basin):
    in_maps, ctx = _prepare(y_pred, y_true, basin)
    res = run_bass_kernel_spmd(_get_nc(), in_maps, list(range(N_CORES)))
    return _finish(res.results, ctx)


# revision 5
# speedup vs baseline: 1.1180x; 1.1180x over previous
"""MeanNSE (segment-reduce) Trainium2 kernel — 8 NeuronCores, data-parallel.

v2: PE-array segment reduction. The basin ids are pure index data, so all
index math runs on the host; the device does every FLOP over the 16.7M
float arrays.

Host: stable-sort elements by basin and pack them into per-core [128, C]
(C = 17408 = 34*512) bf16 tiles in "pillar slot" layout: slot s in [0,512)
owns the 4352 elements at positions {(p, g*512 + s) : p<128, g<34}; each
basin is padded (with zeros) to a whole number of slots, so every slot
contains elements of exactly one basin.  Slot sums can then be computed on
the TENSOR engine: a [128,1] ones stationary x [128,512] moving matmul
yields all 512 per-column partition-sums of one 512-col chunk, and the 34
chunk matmuls accumulate in one PSUM bank (per-element has_written logic),
producing Sum over each slot's full pillar.

Device (per core), three stats per slot, engines balanced:
  - TENSOR: 3 stat passes x 34 accumulating matmuls (N=512, ones
    stationary, ~0.42 ns/col) into 3 psum banks; ~40 warm-up matmuls into a
    scratch bank during the initial DMA keep the PE HAM clock at 2.4 GHz.
  - DVE:  d = t - p, d2 = d*d   (tensor_tensor bf16 runs in 2x mode)
  - ACT:  t2 = Square(t)        (activation, 1x)
  - DMA: inputs stream over sync + scalar HW DGE queues and the gpsimd
    software queue, byte-balanced (~3MB each), small tiles first/last.
Outputs are just 3x512 f32 slot sums -> psum drained via DVE/ACT copies
-> one tiny DMA out.

Host: bincount slot sums back to basins (slot->basin map is host data),
combine in float64 with exact integer counts:
  ss_tot = sum_t2 - sum_t^2/count, nse = 1 - ss_res/(ss_tot + 1e-10),
  answer = mean over 671 basins.
"""

import sys

sys.path.insert(0, "/opt/trn_rl_repo")

import numpy as np
import ml_dtypes

import concourse.bacc as bacc
import concourse.mybir as mybir
import concourse.tile as tile
from concourse.bass_utils import run_bass_kernel_spmd

F32 = mybir.dt.float32
BF16 = mybir.dt.bfloat16
BF16_NP = ml_dtypes.bfloat16

N_CORES = 8
N_TOTAL = 16777216
N_BASINS = 671
EPS = 1e-10

P = 128  # partitions
CH = 512  # psum bank width (f32) = matmul N
SUP = 34  # chunks per stat pass
C = SUP * CH  # columns per core (17408)
E_C = P * C  # elements per core (2,228,224)
PILLAR = P * SUP  # elements per slot (4352)
SLOTS = CH  # slots per core (512)
U_TOT = N_CORES * SLOTS  # global slot-units (4096)

# DMA tile plan, in 512-col chunks (sums to SUP=34): small tiles first so
# compute starts early, small tiles last so the tail drains fast.
K_PLAN = [1, 1, 2, 4, 4, 4, 4, 4, 4, 4, 1, 1]
N_WARM = 8  # PE warm-up matmuls bridging preamble -> first data

_AF = mybir.ActivationFunctionType

_cache = {}


def _dma_schedule():
    """Tile-major round-robin: the sim's DMA device is a single serialized
    resource ordered by trigger time, so interleaving (yt0, yp0, yt1, ...)
    across the three trigger engines makes tiles land in exact tile order,
    with each tile's yt/yp adjacent."""
    qnames = ["gpsimd", "sync", "scalar"]
    sched = {q: [] for q in qnames}
    order = []
    for t in range(len(K_PLAN)):
        order.append(("yt", t))
        order.append(("yp", t))
    for i, pair in enumerate(order):
        sched[qnames[i % 3]].append(pair)
    return sched


def _build():
    nc = bacc.Bacc()
    yt = nc.declare_dram_parameter("yt", [E_C], BF16, isOutput=False)
    yp = nc.declare_dram_parameter("yp", [E_C], BF16, isOutput=False)
    # out: [sum_t(512) | sum_t2(512) | sum_d2(512)]
    out = nc.declare_dram_parameter("out", [3 * SLOTS], F32, isOutput=True)

    yt2d = yt[:].rearrange("(p c) -> p c", p=P, c=C)
    yp2d = yp[:].rearrange("(p c) -> p c", p=P, c=C)

    sched = _dma_schedule()

    with tile.TileContext(nc) as tc:
        with (
            tc.tile_pool(name="const", bufs=1) as cpool,
            tc.tile_pool(name="io", bufs=1) as io_pool,
            tc.tile_pool(name="dx", bufs=3) as d_pool,
            tc.tile_pool(name="d2x", bufs=4) as d2_pool,
            tc.tile_pool(name="t2x", bufs=4) as t2_pool,
            tc.tile_pool(name="ps", bufs=1, space="PSUM") as psum_pool,
        ):
            ones = cpool.tile([P, 1], BF16, tag="ones")
            warm = cpool.tile([P, CH], BF16, tag="warm")
            outs = cpool.tile([1, 3 * SLOTS], F32, tag="outs")
            nc.vector.memset(ones[:, :], 1.0)
            nc.vector.memset(warm[:, :], 0.0)

            p_t = psum_pool.tile([1, CH], F32, tag="p_t")
            p_t2 = psum_pool.tile([1, CH], F32, tag="p_t2")
            p_d2 = psum_pool.tile([1, CH], F32, tag="p_d2")
            p_w = psum_pool.tile([1, CH], F32, tag="p_w")

            # PE warm-up: keep the HAM activity monitor busy during the
            # initial DMA so real matmuls run at 2.4 GHz.
            for w in range(N_WARM):
                nc.tensor.matmul(
                    p_w[:, :], ones[:, :], warm[:, :],
                    start=(w == 0), stop=(w == N_WARM - 1),
                )

            # stage all input tiles up front across the three DMA queues
            tiles = []
            base = 0
            for t, k in enumerate(K_PLAN):
                tt_ = io_pool.tile([P, k * CH], BF16, tag=f"yt{t}")
                tp_ = io_pool.tile([P, k * CH], BF16, tag=f"yp{t}")
                tiles.append((tt_, tp_, k, base))
                base += k * CH

            def _dst(arr, t):
                return tiles[t][0 if arr == "yt" else 1][:, :]

            def _src(arr, t):
                b, k = tiles[t][3], tiles[t][2]
                src = yt2d if arr == "yt" else yp2d
                return src[:, b : b + k * CH]

            for arr, t in sched["gpsimd"]:
                nc.gpsimd.dma_start(_dst(arr, t), _src(arr, t))
            for arr, t in sched["sync"]:
                nc.sync.dma_start(_dst(arr, t), _src(arr, t))
            for arr, t in sched["scalar"]:
                nc.scalar.dma_start(_dst(arr, t), _src(arr, t))

            # PE consumes the quadratic stats one tile behind the raw-t
            # stream, so it never head-of-line blocks on DVE/ACT results.
            def _mm(psum, src, cg0, k):
                for c in range(k):
                    nc.tensor.matmul(
                        psum[:, :], ones[:, :], src[:, c * CH : (c + 1) * CH],
                        start=(cg0 + c == 0), stop=(cg0 + c == SUP - 1),
                    )

            lagged = []  # (t2_tile, d2_tile, cg0, k) awaiting PE
            cg = 0
            for tt_, tp_, k, base in tiles:
                # raw-t slot sums: only need yt, keep PE fed right away
                _mm(p_t, tt_, cg, k)
                # t^2 on ACT (depends only on yt)
                t2_t = t2_pool.tile([P, k * CH], BF16, tag="t2")
                nc.scalar.activation(t2_t[:, :], tt_[:, :], _AF.Square)
                # (t-p)^2 on DVE (both tensor_tensor ops run 2x in bf16)
                d_t = d_pool.tile([P, k * CH], BF16, tag="d")
                nc.vector.tensor_sub(d_t[:, :], tt_[:, :], tp_[:, :])
                d2_t = d2_pool.tile([P, k * CH], BF16, tag="d2")
                nc.vector.tensor_mul(d2_t[:, :], d_t[:, :], d_t[:, :])
                lagged.append((t2_t, d2_t, cg, k))
                if len(lagged) > 1:
                    lt2, ld2, lcg, lk = lagged.pop(0)
                    _mm(p_t2, lt2, lcg, lk)
                    _mm(p_d2, ld2, lcg, lk)
                cg += k

            # p_t is complete: drain it on ACT while PE flushes the last tile
            nc.scalar.activation(outs[:, 0:CH], p_t[:, :], _AF.Copy)
            lt2, ld2, lcg, lk = lagged.pop(0)
            _mm(p_d2, ld2, lcg, lk)
            _mm(p_t2, lt2, lcg, lk)
            # drain remaining psum banks on two engines in parallel
            nc.vector.tensor_copy(outs[:, 2 * CH : 3 * CH], p_d2[:, :])
            nc.scalar.activation(outs[:, CH : 2 * CH], p_t2[:, :], _AF.Copy)

            nc.sync.dma_start(
                out[:].rearrange("(p x) -> p x", p=1, x=3 * SLOTS),
                outs[:, :],
            )
    nc.compile()
    return nc


def _get_nc():
    if "nc" not in _cache:
        _cache["nc"] = _build()
    return _cache["nc"]


def _prepare(y_pred, y_true, basin):
    """Host-side index math: sort by basin, pack into pillar-slot layout."""
    y_pred = np.asarray(y_pred, dtype=np.float32)
    y_true = np.asarray(y_true, dtype=np.float32)
    b = np.asarray(basin).astype(np.int32)
    n = b.shape[0]

    counts = np.bincount(b, minlength=N_BASINS)
    m = (counts + PILLAR - 1) // PILLAR  # slots per basin
    u_tot = int(m.sum())
    assert u_tot <= U_TOT, (u_tot, U_TOT)
    base_u = np.zeros(N_BASINS + 1, np.int64)
    np.cumsum(m, out=base_u[1:])

    order = np.argsort(b, kind="stable")
    seg_start = np.zeros(N_BASINS, np.int64)
    np.cumsum(counts[:-1], out=seg_start[1:])
    bs = b[order]
    i_local = np.arange(n, dtype=np.int64) - seg_start[bs]
    su = base_u[bs] + i_local // PILLAR  # global slot-unit
    j = i_local % PILLAR
    p = j // SUP
    g = j % SUP
    core = su // SLOTS
    s = su % SLOTS
    dst = core * E_C + p * C + g * CH + s

    yt_pad = np.zeros(N_CORES * E_C, dtype=BF16_NP)
    yp_pad = np.zeros(N_CORES * E_C, dtype=BF16_NP)
    yt_pad[dst] = y_true[order].astype(BF16_NP)
    yp_pad[dst] = y_pred[order].astype(BF16_NP)
    yt_pad = yt_pad.reshape(N_CORES, E_C)
    yp_pad = yp_pad.reshape(N_CORES, E_C)

    in_maps = [{"yt": yt_pad[c], "yp": yp_pad[c]} for c in range(N_CORES)]

    # basin of every global slot-unit (pad units -> N_BASINS, dropped later)
    slot_basin = np.full(U_TOT, N_BASINS, np.int64)
    slot_basin[:u_tot] = np.repeat(np.arange(N_BASINS), m)
    return in_maps, (counts, slot_basin)


def _finish(results, ctx):
    counts, slot_basin = ctx
    sums = np.empty((3, U_TOT), np.float64)
    for c in range(N_CORES):
        arr = np.asarray(results[c]["out"], np.float64).reshape(3, SLOTS)
        sums[:, c * SLOTS : (c + 1) * SLOTS] = arr
    s_t = np.bincount(slot_basin, weights=sums[0], minlength=N_BASINS + 1)[:N_BASINS]
    s_t2 = np.bincount(slot_basin, weights=sums[1], minlength=N_BASINS + 1)[:N_BASINS]
    s_d2 = np.bincount(slot_basin, weights=sums[2], minlength=N_BASINS + 1)[:N_BASINS]
    cnt = counts.astype(np.float64)
    ss_tot = s_t2 - s_t * s_t / cnt
    nse = 1.0 - s_d2 / (ss_tot + EPS)
    return np.float32(nse.mean())


def kernel(y_pred, y_true, basin):
    in_maps, ctx = _prepare(y_pred, y_true, basin)
    res = run_bass_kernel_spmd(_get_nc(), in_maps, list(range(N_CORES)))
    return _finish(res.results, ctx)


# revision 7
# speedup vs baseline: 1.2789x; 1.1440x over previous
"""MeanNSE (segment-reduce) Trainium2 kernel — 8 NeuronCores, data-parallel.

v2: PE-array segment reduction. The basin ids are pure index data, so all
index math runs on the host; the device does every FLOP over the 16.7M
float arrays.

Host: stable-sort elements by basin and pack them into per-core [128, C]
(C = 17408 = 34*512) bf16 tiles in "pillar slot" layout: slot s in [0,512)
owns the 4352 elements at positions {(p, g*512 + s) : p<128, g<34}; each
basin is padded (with zeros) to a whole number of slots, so every slot
contains elements of exactly one basin.  Slot sums can then be computed on
the TENSOR engine: a [128,1] ones stationary x [128,512] moving matmul
yields all 512 per-column partition-sums of one 512-col chunk, and the 34
chunk matmuls accumulate in one PSUM bank (per-element has_written logic),
producing Sum over each slot's full pillar.

Device (per core), three stats per slot, engines balanced:
  - TENSOR: 3 stat passes x 34 accumulating matmuls (N=512, ones
    stationary, ~0.42 ns/col) into 3 psum banks; ~40 warm-up matmuls into a
    scratch bank during the initial DMA keep the PE HAM clock at 2.4 GHz.
  - DVE:  d = t - p, d2 = d*d   (tensor_tensor bf16 runs in 2x mode)
  - ACT:  t2 = Square(t)        (activation, 1x)
  - DMA: inputs stream over sync + scalar HW DGE queues and the gpsimd
    software queue, byte-balanced (~3MB each), small tiles first/last.
Outputs are just 3x512 f32 slot sums -> psum drained via DVE/ACT copies
-> one tiny DMA out.

Host: bincount slot sums back to basins (slot->basin map is host data),
combine in float64 with exact integer counts:
  ss_tot = sum_t2 - sum_t^2/count, nse = 1 - ss_res/(ss_tot + 1e-10),
  answer = mean over 671 basins.
"""

import sys

sys.path.insert(0, "/opt/trn_rl_repo")

import numpy as np
import ml_dtypes

import concourse.bacc as bacc
import concourse.mybir as mybir
import concourse.tile as tile
from concourse.bass_utils import run_bass_kernel_spmd

F32 = mybir.dt.float32
BF16 = mybir.dt.bfloat16
BF16_NP = ml_dtypes.bfloat16

N_CORES = 8
N_TOTAL = 16777216
N_BASINS = 671
EPS = 1e-10

P = 128  # partitions
CH = 512  # psum bank width (f32) = matmul N
SUP = 34  # chunks per stat pass
C = SUP * CH  # columns per core (17408)
E_C = P * C  # elements per core (2,228,224)
PILLAR = P * SUP  # elements per slot (4352)
SLOTS = CH  # slots per core (512)
U_TOT = N_CORES * SLOTS  # global slot-units (4096)

# DMA tile plan, in 512-col chunks (sums to SUP=34): small tiles first so
# compute starts early, small tiles last so the tail drains fast.
K_PLAN = [1, 1, 2, 4, 4, 4, 4, 4, 4, 4, 1, 1]
N_WARM = 5  # PE warm-up matmuls bridging preamble -> first data

_AF = mybir.ActivationFunctionType

_cache = {}


def _dma_schedule():
    """Each HW DGE engine queue holds only 4 outstanding DMAs — the 5th
    trigger blocks that engine's sequencer until an earlier DMA completes.
    So scalar (which must run the squares) gets exactly 4 early small tiles;
    sync (otherwise idle) and gpsimd (software DGE, non-blocking triggers)
    alternate the rest so tiles land roughly in tile order."""
    sched = {"scalar": [("yt", 0), ("yp", 0), ("yt", 1), ("yp", 1)],
             "sync": [], "gpsimd": []}
    for t in range(2, len(K_PLAN)):
        a, b = ("sync", "gpsimd") if t % 2 == 0 else ("gpsimd", "sync")
        sched[a].append(("yt", t))
        sched[b].append(("yp", t))
    return sched


def _build():
    nc = bacc.Bacc()
    yt = nc.declare_dram_parameter("yt", [E_C], BF16, isOutput=False)
    yp = nc.declare_dram_parameter("yp", [E_C], BF16, isOutput=False)
    # out: [sum_t(512) | sum_t2(512) | sum_d2(512)]
    out = nc.declare_dram_parameter("out", [3 * SLOTS], F32, isOutput=True)

    yt2d = yt[:].rearrange("(p c) -> p c", p=P, c=C)
    yp2d = yp[:].rearrange("(p c) -> p c", p=P, c=C)

    sched = _dma_schedule()

    with tile.TileContext(nc) as tc:
        with (
            tc.tile_pool(name="const", bufs=1) as cpool,
            tc.tile_pool(name="io", bufs=1) as io_pool,
            tc.tile_pool(name="dx", bufs=3) as d_pool,
            tc.tile_pool(name="d2x", bufs=4) as d2_pool,
            tc.tile_pool(name="t2x", bufs=4) as t2_pool,
            tc.tile_pool(name="ps", bufs=1, space="PSUM") as psum_pool,
        ):
            ones = cpool.tile([P, 1], BF16, tag="ones")
            warm = cpool.tile([P, CH], BF16, tag="warm")
            outs = cpool.tile([1, 3 * SLOTS], F32, tag="outs")
            nc.vector.memset(ones[:, :], 1.0)
            nc.vector.memset(warm[:, :], 0.0)

            p_t = psum_pool.tile([1, CH], F32, tag="p_t")
            p_t2 = psum_pool.tile([1, CH], F32, tag="p_t2")
            p_d2 = psum_pool.tile([1, CH], F32, tag="p_d2")
            p_w = psum_pool.tile([1, CH], F32, tag="p_w")

            # PE warm-up: keep the HAM activity monitor busy during the
            # initial DMA so real matmuls run at 2.4 GHz.
            for w in range(N_WARM):
                nc.tensor.matmul(
                    p_w[:, :], ones[:, :], warm[:, :],
                    start=(w == 0), stop=(w == N_WARM - 1),
                )

            # stage all input tiles up front across the three DMA queues
            tiles = []
            base = 0
            for t, k in enumerate(K_PLAN):
                tt_ = io_pool.tile([P, k * CH], BF16, tag=f"yt{t}")
                tp_ = io_pool.tile([P, k * CH], BF16, tag=f"yp{t}")
                tiles.append((tt_, tp_, k, base))
                base += k * CH

            def _dst(arr, t):
                return tiles[t][0 if arr == "yt" else 1][:, :]

            def _src(arr, t):
                b, k = tiles[t][3], tiles[t][2]
                src = yt2d if arr == "yt" else yp2d
                return src[:, b : b + k * CH]

            for arr, t in sched["gpsimd"]:
                nc.gpsimd.dma_start(_dst(arr, t), _src(arr, t))
            for arr, t in sched["sync"]:
                nc.sync.dma_start(_dst(arr, t), _src(arr, t))
            for arr, t in sched["scalar"]:
                nc.scalar.dma_start(_dst(arr, t), _src(arr, t))

            # PE consumes the quadratic stats one tile behind the raw-t
            # stream, so it never head-of-line blocks on DVE/ACT results.
            def _mm(psum, src, cg0, k):
                for c in range(k):
                    nc.tensor.matmul(
                        psum[:, :], ones[:, :], src[:, c * CH : (c + 1) * CH],
                        start=(cg0 + c == 0), stop=(cg0 + c == SUP - 1),
                    )

            lagged = []  # (t2_tile, d2_tile, cg0, k) awaiting PE
            cg = 0
            for tt_, tp_, k, base in tiles:
                # raw-t slot sums: only need yt, keep PE fed right away
                _mm(p_t, tt_, cg, k)
                # t^2 on ACT (depends only on yt)
                t2_t = t2_pool.tile([P, k * CH], BF16, tag="t2")
                nc.scalar.activation(t2_t[:, :], tt_[:, :], _AF.Square)
                # (t-p)^2 on DVE (both tensor_tensor ops run 2x in bf16)
                d_t = d_pool.tile([P, k * CH], BF16, tag="d")
                nc.vector.tensor_sub(d_t[:, :], tt_[:, :], tp_[:, :])
                d2_t = d2_pool.tile([P, k * CH], BF16, tag="d2")
                nc.vector.tensor_mul(d2_t[:, :], d_t[:, :], d_t[:, :])
                lagged.append((t2_t, d2_t, cg, k))
                if len(lagged) > 1:
                    lt2, ld2, lcg, lk = lagged.pop(0)
                    _mm(p_t2, lt2, lcg, lk)
                    _mm(p_d2, ld2, lcg, lk)
                cg += k

            # p_t is complete: drain it on ACT while PE flushes the last tile
            nc.scalar.activation(outs[:, 0:CH], p_t[:, :], _AF.Copy)
            lt2, ld2, lcg, lk = lagged.pop(0)
            _mm(p_d2, ld2, lcg, lk)
            _mm(p_t2, lt2, lcg, lk)
            # drain remaining psum banks on two engines in parallel
            nc.vector.tensor_copy(outs[:, 2 * CH : 3 * CH], p_d2[:, :])
            nc.scalar.activation(outs[:, CH : 2 * CH], p_t2[:, :], _AF.Copy)

            nc.sync.dma_start(
                out[:].rearrange("(p x) -> p x", p=1, x=3 * SLOTS),
                outs[:, :],
            )
    nc.compile()
    return nc


def _get_nc():
    if "nc" not in _cache:
        _cache["nc"] = _build()
    return _cache["nc"]


def _prepare(y_pred, y_true, basin):
    """Host-side index math: sort by basin, pack into pillar-slot layout."""
    y_pred = np.asarray(y_pred, dtype=np.float32)
    y_true = np.asarray(y_true, dtype=np.float32)
    b = np.asarray(basin).astype(np.int32)
    n = b.shape[0]

    counts = np.bincount(b, minlength=N_BASINS)
    m = (counts + PILLAR - 1) // PILLAR  # slots per basin
    u_tot = int(m.sum())
    assert u_tot <= U_TOT, (u_tot, U_TOT)
    base_u = np.zeros(N_BASINS + 1, np.int64)
    np.cumsum(m, out=base_u[1:])

    order = np.argsort(b, kind="stable")
    seg_start = np.zeros(N_BASINS, np.int64)
    np.cumsum(counts[:-1], out=seg_start[1:])
    bs = b[order]
    i_local = np.arange(n, dtype=np.int64) - seg_start[bs]
    su = base_u[bs] + i_local // PILLAR  # global slot-unit
    j = i_local % PILLAR
    p = j // SUP
    g = j % SUP
    core = su // SLOTS
    s = su % SLOTS
    dst = core * E_C + p * C + g * CH + s

    yt_pad = np.zeros(N_CORES * E_C, dtype=BF16_NP)
    yp_pad = np.zeros(N_CORES * E_C, dtype=BF16_NP)
    yt_pad[dst] = y_true[order].astype(BF16_NP)
    yp_pad[dst] = y_pred[order].astype(BF16_NP)
    yt_pad = yt_pad.reshape(N_CORES, E_C)
    yp_pad = yp_pad.reshape(N_CORES, E_C)

    in_maps = [{"yt": yt_pad[c], "yp": yp_pad[c]} for c in range(N_CORES)]

    # basin of every global slot-unit (pad units -> N_BASINS, dropped later)
    slot_basin = np.full(U_TOT, N_BASINS, np.int64)
    slot_basin[:u_tot] = np.repeat(np.arange(N_BASINS), m)
    return in_maps, (counts, slot_basin)


def _finish(results, ctx):
    counts, slot_basin = ctx
    sums = np.empty((3, U_TOT), np.float64)
    for c in range(N_CORES):
        arr = np.asarray(results[c]["out"], np.float64).reshape(3, SLOTS)
        sums[:, c * SLOTS : (c + 1) * SLOTS] = arr
    s_t = np.bincount(slot_basin, weights=sums[0], minlength=N_BASINS + 1)[:N_BASINS]
    s_t2 = np.bincount(slot_basin, weights=sums[1], minlength=N_BASINS + 1)[:N_BASINS]
    s_d2 = np.bincount(slot_basin, weights=sums[2], minlength=N_BASINS + 1)[:N_BASINS]
    cnt = counts.astype(np.float64)
    ss_tot = s_t2 - s_t * s_t / cnt
    nse = 1.0 - s_d2 / (ss_tot + EPS)
    return np.float32(nse.mean())


def kernel(y_pred, y_true, basin):
    in_maps, ctx = _prepare(y_pred, y_true, basin)
    res = run_bass_kernel_spmd(_get_nc(), in_maps, list(range(N_CORES)))
    return _finish(res.results, ctx)
